# revision 27
# baseline (speedup 1.0000x reference)
"""AdderNet BasicBlock kernel for Trainium2, co-sharded across 8 cores.

Per core (co-shard CO=8 of 64 output channels):
  conv[co,n,p] = -sum_{ci,kh,kw} |x[n,ci,p+k-1] - w[co,ci,kh,kw]|   (pad=1)
  BN train-mode over (n,h,w) per co, then ReLU.

Formulation: |d| = 2*relu(d) - d with d = x - w, so
  conv = -2*sum(relu(x-w)) + BoxX - S_w
    BoxX = sum_{ci,tap} x_patch  (PE ones-matmuls on x directly)
    S_w  = sum_{ci,tap} w[co]    (precomputed on host, applied in stage 2)

Design:
  - x is pre-padded / pre-transposed / pre-bf16 on the host, in two copies
    (xb shifted by one element) so every tap view is 4B-aligned and the DVE
    tensor_scalar relu runs in 4x mode; ACT takes a tuned fraction.
  - 4-way column-tiled concurrent PE reduce: rounds of 4 streams map to PE
    column strips 0/32/64/96 (tile_position); each stream accumulates its 9
    taps into rows 32j:32j+2 of a shared [128,4096] PSUM tile.
  - sel weights are -2 for conv streams (+1 for box), so PSUM holds the
    -2*sum(relu) term directly; evacuation is one ACT copy per round and
    SBUF->SBUF DMAs redistribute straight into the stage-2 layout
    [(co,n), hw] (no DRAM bounce).
  - stage 2 uses accum_out / tensor_tensor_reduce so BN stats need only
    two tiny N=1 matmuls for the cross-partition sums.
"""
from contextlib import ExitStack

import numpy as np

import concourse.bass as bass
import concourse.tile as tile
import concourse.mybir as mybir

F32 = mybir.dt.float32
BF16 = mybir.dt.bfloat16
F32R = mybir.dt.float32r
BN_EPS = 1e-5

N, CI, H, W = 16, 64, 32, 32
CO = 8          # output channels per core
HW = H * W      # 1024
PADH, PADW = H + 2, W + 2  # 34
JPH = 4         # images per group per half
NH = 2          # halves (JPH*NH*2groups = 16 images)
TCOLS = JPH * HW            # 4096 free cols per tap tile
NB = TCOLS // 512           # 512-col psum blocks


def split_multiwaits(nc, max_waits=1):
    """This container's walrus rejects >1 semaphore wait per instruction.
    Hoist extras into standalone NoOps on the same (in-order) engine."""
    n_split = 0
    for f in nc.m.functions:
        for b in f.blocks:
            insts = list(b.instructions)
            changed = False
            new = []
            for inst in insts:
                si = inst.sync_info
                waits = list(si.on_wait) if si and si.on_wait else []
                if len(waits) > max_waits:
                    changed = True
                    n_split += 1
                    for w in waits[: len(waits) - max_waits]:
                        new.append(mybir.InstNoOp(
                            name=nc.get_next_instruction_name(),
                            engine=inst.engine, ins=[], outs=[],
                            sync_info=mybir.SyncInfo(on_wait=[w], on_update=[]),
                        ))
                    inst.sync_info = mybir.SyncInfo(
                        on_wait=waits[len(waits) - max_waits:],
                        on_update=list(si.on_update) if si.on_update else [],
                    )
                new.append(inst)
            if changed:
                b.instructions = new
    return n_split


def build_nc(act_frac=0.23, debug_out=None):
    """One core's SPMD program."""
    nc = bass.Bass()
    xa = nc.declare_dram_parameter("xa", [CI, N, PADH, PADW], BF16,
                                   isOutput=False)
    xb = nc.declare_dram_parameter("xb", [CI, N, PADH, W], BF16,
                                   isOutput=False)
    w = nc.declare_dram_parameter("w", [CO, CI, 3, 3], F32, isOutput=False)
    wneg = nc.declare_dram_parameter("wneg", [CO, CI, 3, 3], F32,
                                     isOutput=False)
    swneg = nc.declare_dram_parameter("swneg", [CO], F32, isOutput=False)
    gamma = nc.declare_dram_parameter("gamma", [CO], F32, isOutput=False)
    beta = nc.declare_dram_parameter("beta", [CO], F32, isOutput=False)
    selcor_in = nc.declare_dram_parameter("selcor", [128, 128], F32,
                                          isOutput=False)
    out = nc.declare_dram_parameter("out", [N, CO, H, W], F32, isOutput=True)

    # stream list: 9 per half (1 box + 8 conv channels); rounds of 4
    streams = []
    for h in range(NH):
        streams.append(("box", None, h))
        for co in range(CO):
            streams.append(("co", co, h))
    n_rounds = (len(streams) + 3) // 4  # 5 (last round has 2 streams)

    with tile.TileContext(nc) as tc, ExitStack() as ctx:
        singles = ctx.enter_context(tc.tile_pool(name="singles", bufs=1))
        tpool = ctx.enter_context(tc.tile_pool(name="tpool", bufs=12))
        cpool = ctx.enter_context(tc.tile_pool(name="cpool", bufs=2))
        pspool = ctx.enter_context(tc.tile_pool(name="psumA", bufs=1,
                                                space="PSUM"))
        pspoolB = ctx.enter_context(tc.tile_pool(name="psumB", bufs=1,
                                                 space="PSUM"))
        spool = ctx.enter_context(tc.tile_pool(name="stage2", bufs=1))

        # ---- x first (big DMAs; first round starts as soon as half 0 lands)
        aux0s, aux1s = [], []
        for half in range(NH):
            j0 = half * JPH
            a0 = singles.tile([128, JPH, PADH, PADW], BF16, name=f"a0_{half}")
            a1 = singles.tile([128, JPH, PADH, W], BF16, name=f"a1_{half}")
            for g in range(2):
                nc.sync.dma_start(out=a0[g * 64:(g + 1) * 64],
                                  in_=xa[:, g * 8 + j0:g * 8 + j0 + JPH])
                nc.sync.dma_start(out=a1[g * 64:(g + 1) * 64],
                                  in_=xb[:, g * 8 + j0:g * 8 + j0 + JPH])
            aux0s.append(a0)
            aux1s.append(a1)

        # ---- weights (needed by the first tap tiles) ----
        w_sb = singles.tile([128, CO * 9], F32)
        neg_w_sb = singles.tile([128, CO * 9], F32)
        w_src = w.rearrange("co ci kh kw -> ci co (kh kw)")
        wneg_src = wneg.rearrange("co ci kh kw -> ci co (kh kw)")
        for g in range(2):
            nc.sync.dma_start(
                out=w_sb[g * 64:(g + 1) * 64, :].rearrange(
                    "p (co t) -> p co t", t=9), in_=w_src)
            nc.sync.dma_start(
                out=neg_w_sb[g * 64:(g + 1) * 64, :].rearrange(
                    "p (co t) -> p co t", t=9), in_=wneg_src)

        # ---- PE selector weights (bf16, exact): -2 * group-reduce for ALL
        # streams (box too; fixed up with a -0.5 scale in stage 2) so the
        # stationary weights never change.
        selm2 = singles.tile([128, 2], BF16)
        nc.vector.memset(selm2[:, :], 0.0)
        nc.vector.memset(selm2[0:64, 0:1], -2.0)
        nc.vector.memset(selm2[64:128, 1:2], -2.0)
        eps_t = singles.tile([128, 1], F32)
        nc.vector.memset(eps_t[:, :], BN_EPS)

        gam = singles.tile([128, 1], F32)
        bet = singles.tile([128, 1], F32)
        for co in range(CO):
            nc.sync.dma_start(out=gam[co * 16:(co + 1) * 16, :],
                              in_=gamma[co:co + 1].partition_broadcast(16))
            nc.sync.dma_start(out=bet[co * 16:(co + 1) * 16, :],
                              in_=beta[co:co + 1].partition_broadcast(16))
        selcor = singles.tile([128, 128], F32)      # replicated stats selector
        nc.sync.dma_start(out=selcor[:, :], in_=selcor_in[:, :])

        def tap_src(half, kh, kw):
            """bf16 view of the (kh,kw)-shifted window, 4B-aligned."""
            if kw == 1:
                return aux1s[half][:, :, kh:kh + H, 0:W]
            return aux0s[half][:, :, kh:kh + H, kw:kw + W]

        def box_rhs(half, kh, kw, b):
            a, hb = divmod(b, 2)
            if kw == 1:
                return aux1s[half][:, a, kh + hb * 16:kh + hb * 16 + 16, 0:W]
            return aux0s[half][:, a, kh + hb * 16:kh + hb * 16 + 16,
                               kw:kw + W]

        # conv scratch in DRAM (partition-crossing redistribution)
        dpool = ctx.enter_context(tc.tile_pool(name="dram", bufs=1,
                                               space="DRAM"))
        conv_d = dpool.tile([CO, 2, NH, JPH, HW], F32)
        box_d = dpool.tile([2, NH, JPH, HW], F32)

        # stage-2 reload targets, loaded incrementally
        cs_rl = spool.tile([128, HW], F32)      # [(co,n), hw] = -2*sum(relu)
        box_rl = spool.tile([128, HW], F32)     # BoxX broadcast per co

        # ---- stage 1: rounds of up to 4 concurrent streams ----
        acc = 0.0
        co_last_round = {}
        box_last_round = 0
        for s, (kind, co, h) in enumerate(streams):
            if kind == "co":
                co_last_round[co] = s // 4
            else:
                box_last_round = s // 4
        HB = TCOLS // 2  # psum half-tile cols (2048)
        for r in range(n_rounds):
            rs = streams[4 * r:4 * r + 4]
            # two psum half-tiles (images 0-1 / 2-3): next round can start
            # in half A while half B is still evacuating
            psA = pspool.tile([128, HB], F32, tag="ps", name=f"psA_{r}")
            psB = pspoolB.tile([128, HB], F32, tag="psb", name=f"psB_{r}")

            def emit_mm(tap, b_range):
                kh, kw = divmod(tap, 3)
                for b in b_range:
                    ps = psA if b < NB // 2 else psB
                    col = (b % (NB // 2)) * 512
                    for j, (kind, co, h) in enumerate(rs):
                        if kind == "co":
                            rhs = t_tiles[j][:, b * 512:(b + 1) * 512]
                        else:
                            rhs = box_rhs(h, kh, kw, b)
                        nc.tensor.matmul(
                            ps[32 * j:32 * j + 2, col:col + 512],
                            lhsT=selm2[:, :], rhs=rhs,
                            start=(tap == 0), stop=(tap == 8),
                            tile_position=(0, 32 * j))

            t_tiles = {}
            for tap in range(9):
                kh, kw = divmod(tap, 3)
                for j, (kind, co, h) in enumerate(rs):
                    if kind != "co":
                        continue
                    t = tpool.tile([128, JPH, H, W], BF16, tag="t",
                                   name=f"t_{r}_{j}_{tap}")
                    src = tap_src(h, kh, kw)
                    k = co * 9 + tap
                    acc += act_frac
                    if acc >= 1.0:
                        acc -= 1.0
                        nc.scalar.activation(
                            out=t[:, :, :, :], in_=src,
                            func=mybir.ActivationFunctionType.Relu,
                            bias=neg_w_sb[:, k:k + 1], scale=1.0)
                    else:
                        nc.vector.tensor_scalar(
                            out=t[:, :, :, :], in0=src,
                            scalar1=w_sb[:, k:k + 1], scalar2=0.0,
                            op0=mybir.AluOpType.subtract,
                            op1=mybir.AluOpType.max)
                    t_tiles[j] = t.rearrange("p a h w -> p (a h w)")
                if tap < 8:
                    emit_mm(tap, range(NB))
                else:
                    emit_mm(tap, range(NB // 2))

            csA = cpool.tile([128, HB], F32, tag="cs", name=f"csA_{r}")
            nc.scalar.copy(csA[:, :], psA[:, :])
            emit_mm(8, range(NB // 2, NB))
            csB = cpool.tile([128, HB], F32, tag="cs", name=f"csB_{r}")
            nc.scalar.copy(csB[:, :], psB[:, :])
            for j, (kind, co, h) in enumerate(rs):
                for half_i, cs in enumerate((csA, csB)):
                    strip = cs[32 * j:32 * j + 2, :].rearrange(
                        "p (a hw) -> p a hw", hw=HW)
                    asl = slice(half_i * 2, half_i * 2 + 2)
                    if kind == "co":
                        nc.sync.dma_start(out=conv_d[co, :, h, asl, :],
                                          in_=strip)
                    else:
                        nc.sync.dma_start(out=box_d[:, h, asl, :], in_=strip)
            # incremental stage-2 reloads as soon as a channel completes
            for co2, lr in co_last_round.items():
                if lr == r:
                    nc.sync.dma_start(
                        out=cs_rl[co2 * 16:(co2 + 1) * 16, :],
                        in_=conv_d[co2].rearrange("g h j w -> (g h j) w"))
            if r == box_last_round:
                for c2 in range(CO):
                    nc.sync.dma_start(
                        out=box_rl[c2 * 16:(c2 + 1) * 16, :],
                        in_=box_d.rearrange("g h j w -> (g h j) w"))

        # ---- stage 2: BN stats + affine + relu ----
        # conv0 = cs_rl - 0.5*box_rl = conv + S_w (the S_w shift cancels in
        # BN: using mean0 = mean(conv0) everywhere is equivalent), with free
        # per-partition row sums via accum_out
        conv_rl = spool.tile([128, HW], F32)
        psums = spool.tile([128, 1], F32)
        nc.vector.scalar_tensor_tensor(
            out=conv_rl[:, :], in0=box_rl[:, :], scalar=-0.5,
            in1=cs_rl[:, :], op0=mybir.AluOpType.mult,
            op1=mybir.AluOpType.add, accum_out=psums[:, :])

        # second moment per partition, independent of mean (runs right after
        # conv_rl, before the stats matmuls)
        sq = spool.tile([128, HW], F32)
        pssq = spool.tile([128, 1], F32)
        nc.vector.scalar_tensor_tensor(
            out=sq[:, :], in0=conv_rl[:, :], scalar=1.0, in1=conv_rl[:, :],
            op0=mybir.AluOpType.mult, op1=mybir.AluOpType.mult,
            accum_out=pssq[:, :])
        # selcor carries 1/(N*HW), so these give mean / E[c^2] directly
        mean_ps = pspool.tile([128, 1], F32, tag="ps")
        nc.tensor.matmul(mean_ps[:, :], lhsT=selcor[:, :], rhs=psums[:, :],
                         start=True, stop=True)
        ps2 = pspoolB.tile([128, 1], F32, tag="psb")
        nc.tensor.matmul(ps2[:, :], lhsT=selcor[:, :], rhs=pssq[:, :],
                         start=True, stop=True)
        mean = spool.tile([128, 1], F32)
        nc.vector.tensor_copy(out=mean[:, :], in_=mean_ps[:, :])
        # var = E[c^2] - mean^2
        m2 = spool.tile([128, 1], F32)
        nc.vector.tensor_mul(m2[:, :], mean[:, :], mean[:, :])
        var = spool.tile([128, 1], F32)
        nc.vector.tensor_sub(var[:, :], ps2[:, :], m2[:, :])
        std = spool.tile([128, 1], F32)
        nc.scalar.activation(out=std[:, :], in_=var[:, :],
                             func=mybir.ActivationFunctionType.Sqrt,
                             bias=eps_t[:, :], scale=1.0)
        rstd = spool.tile([128, 1], F32)
        nc.vector.reciprocal(out=rstd[:, :], in_=std[:, :])
        a_t = spool.tile([128, 1], F32)
        nc.vector.tensor_mul(a_t[:, :], gam[:, :], rstd[:, :])
        ma = spool.tile([128, 1], F32)
        nc.vector.tensor_mul(ma[:, :], mean[:, :], a_t[:, :])
        b_t = spool.tile([128, 1], F32)
        nc.vector.tensor_sub(b_t[:, :], bet[:, :], ma[:, :])

        outt = spool.tile([128, HW], F32)
        if debug_out == "conv":
            nc.vector.tensor_copy(out=outt[:, :], in_=conv_rl[:, :])
        elif debug_out == "cs":
            nc.vector.tensor_copy(out=outt[:, :], in_=cs_rl[:, :])
        elif debug_out == "box":
            nc.vector.tensor_copy(out=outt[:, :], in_=box_rl[:, :])
        else:
            nc.scalar.activation(out=outt[:, :], in_=conv_rl[:, :],
                                 func=mybir.ActivationFunctionType.Relu,
                                 bias=b_t[:, :], scale=a_t[:, :])
        out_r = out.rearrange("n co h w -> co n (h w)")
        for co in range(CO):
            nc.sync.dma_start(out=out_r[co], in_=outt[co * 16:(co + 1) * 16, :])

    split_multiwaits(nc)
    return nc


def make_in_maps(x, weight, gamma, beta):
    import ml_dtypes
    x = np.ascontiguousarray(x, dtype=np.float32)
    weight = np.ascontiguousarray(weight, dtype=np.float32)
    gamma = np.ascontiguousarray(gamma, dtype=np.float32)
    beta = np.ascontiguousarray(beta, dtype=np.float32)
    # pre-padded, ci-major, bf16 copies of x (xb shifted one element in w)
    xpad = np.zeros((CI, N, PADH, PADW), np.float32)
    xpad[:, :, 1:H + 1, 1:W + 1] = x.transpose(1, 0, 2, 3)
    xa = np.ascontiguousarray(xpad.astype(ml_dtypes.bfloat16))
    xb = np.ascontiguousarray(xa[:, :, :, 1:1 + W])
    selcor = np.zeros((128, 128), np.float32)
    for c in range(CO):
        selcor[c * 16:(c + 1) * 16, c * 16:(c + 1) * 16] = 1.0 / (N * H * W)
    maps = []
    for c in range(8):
        sl = slice(c * CO, (c + 1) * CO)
        wsl = np.ascontiguousarray(weight[sl])
        maps.append({
            "xa": xa,
            "xb": xb,
            "w": wsl,
            "wneg": np.ascontiguousarray(-wsl),
            "swneg": np.ascontiguousarray(-wsl.sum(axis=(1, 2, 3))),
            "gamma": np.ascontiguousarray(gamma[sl]),
            "beta": np.ascontiguousarray(beta[sl]),
            "selcor": selcor,
        })
    return maps


def assemble(results):
    return np.concatenate([r["out"] for r in results], axis=1)


# ---------------------------------------------------------------------------
# Harness entry point: full inputs in, full output out.
# Sharding: output channels co split 8 ways (8 channels per NeuronCore);
# BN statistics are over the full batch, which each core owns for its
# channels, so no collectives are needed.
# ---------------------------------------------------------------------------
from concourse.bass_utils import run_bass_kernel_spmd

_NC_CACHE = None


def _get_nc():
    global _NC_CACHE
    if _NC_CACHE is None:
        _NC_CACHE = build_nc()
    return _NC_CACHE


def kernel(x, weight, gamma, beta):
    nc = _get_nc()
    in_maps = make_in_maps(np.asarray(x), np.asarray(weight),
                           np.asarray(gamma), np.asarray(beta))
    res = run_bass_kernel_spmd(nc, in_maps, core_ids=list(range(8)))
    return assemble(res.results)


# revision 29
# speedup vs baseline: 1.0010x; 1.0010x over previous
"""AdderNet BasicBlock kernel for Trainium2, co-sharded across 8 cores.

Per core (co-shard CO=8 of 64 output channels):
  conv[co,n,p] = -sum_{ci,kh,kw} |x[n,ci,p+k-1] - w[co,ci,kh,kw]|   (pad=1)
  BN train-mode over (n,h,w) per co, then ReLU.

Formulation: |d| = 2*relu(d) - d with d = x - w, so
  conv = -2*sum(relu(x-w)) + BoxX - S_w
    BoxX = sum_{ci,tap} x_patch  (PE ones-matmuls on x directly)
    S_w  = sum_{ci,tap} w[co]    (precomputed on host, applied in stage 2)

Design:
  - x is pre-padded / pre-transposed / pre-bf16 on the host, in two copies
    (xb shifted by one element) so every tap view is 4B-aligned and the DVE
    tensor_scalar relu runs in 4x mode; ACT takes a tuned fraction.
  - 4-way column-tiled concurrent PE reduce: rounds of 4 streams map to PE
    column strips 0/32/64/96 (tile_position); each stream accumulates its 9
    taps into rows 32j:32j+2 of a shared [128,4096] PSUM tile.
  - sel weights are -2 for conv streams (+1 for box), so PSUM holds the
    -2*sum(relu) term directly; evacuation is one ACT copy per round and
    SBUF->SBUF DMAs redistribute straight into the stage-2 layout
    [(co,n), hw] (no DRAM bounce).
  - stage 2 uses accum_out / tensor_tensor_reduce so BN stats need only
    two tiny N=1 matmuls for the cross-partition sums.
"""
from contextlib import ExitStack

import numpy as np

import concourse.bass as bass
import concourse.tile as tile
import concourse.mybir as mybir

F32 = mybir.dt.float32
BF16 = mybir.dt.bfloat16
F32R = mybir.dt.float32r
BN_EPS = 1e-5

N, CI, H, W = 16, 64, 32, 32
CO = 8          # output channels per core
HW = H * W      # 1024
PADH, PADW = H + 2, W + 2  # 34
JPH = 4         # images per group per half
NH = 2          # halves (JPH*NH*2groups = 16 images)
TCOLS = JPH * HW            # 4096 free cols per tap tile
NB = TCOLS // 512           # 512-col psum blocks


def split_multiwaits(nc, max_waits=1):
    """This container's walrus rejects >1 semaphore wait per instruction.
    Hoist extras into standalone NoOps on the same (in-order) engine."""
    n_split = 0
    for f in nc.m.functions:
        for b in f.blocks:
            insts = list(b.instructions)
            changed = False
            new = []
            for inst in insts:
                si = inst.sync_info
                waits = list(si.on_wait) if si and si.on_wait else []
                if len(waits) > max_waits:
                    changed = True
                    n_split += 1
                    for w in waits[: len(waits) - max_waits]:
                        new.append(mybir.InstNoOp(
                            name=nc.get_next_instruction_name(),
                            engine=inst.engine, ins=[], outs=[],
                            sync_info=mybir.SyncInfo(on_wait=[w], on_update=[]),
                        ))
                    inst.sync_info = mybir.SyncInfo(
                        on_wait=waits[len(waits) - max_waits:],
                        on_update=list(si.on_update) if si.on_update else [],
                    )
                new.append(inst)
            if changed:
                b.instructions = new
    return n_split


def build_nc(act_frac=0.23, debug_out=None):
    """One core's SPMD program."""
    nc = bass.Bass()
    xa = nc.declare_dram_parameter("xa", [CI, N, PADH, PADW], BF16,
                                   isOutput=False)
    xb = nc.declare_dram_parameter("xb", [CI, N, PADH, W], BF16,
                                   isOutput=False)
    w = nc.declare_dram_parameter("w", [CO, CI, 3, 3], F32, isOutput=False)
    wneg = nc.declare_dram_parameter("wneg", [CO, CI, 3, 3], F32,
                                     isOutput=False)
    swneg = nc.declare_dram_parameter("swneg", [CO], F32, isOutput=False)
    gamma = nc.declare_dram_parameter("gamma", [CO], F32, isOutput=False)
    beta = nc.declare_dram_parameter("beta", [CO], F32, isOutput=False)
    selcor_in = nc.declare_dram_parameter("selcor", [128, 128], F32,
                                          isOutput=False)
    out = nc.declare_dram_parameter("out", [N, CO, H, W], F32, isOutput=True)

    # stream list: 9 per half (1 box + 8 conv channels); rounds of 4
    streams = []
    for h in range(NH):
        streams.append(("box", None, h))
        for co in range(CO):
            streams.append(("co", co, h))
    n_rounds = (len(streams) + 3) // 4  # 5 (last round has 2 streams)

    with tile.TileContext(nc) as tc, ExitStack() as ctx:
        singles = ctx.enter_context(tc.tile_pool(name="singles", bufs=1))
        tpool = ctx.enter_context(tc.tile_pool(name="tpool", bufs=12))
        cpool = ctx.enter_context(tc.tile_pool(name="cpool", bufs=2))
        pspool = ctx.enter_context(tc.tile_pool(name="psumA", bufs=1,
                                                space="PSUM"))
        pspoolB = ctx.enter_context(tc.tile_pool(name="psumB", bufs=1,
                                                 space="PSUM"))
        spool = ctx.enter_context(tc.tile_pool(name="stage2", bufs=1))

        # ---- x first (big DMAs; first round starts as soon as half 0 lands)
        aux0s, aux1s = [], []
        for half in range(NH):
            j0 = half * JPH
            a0 = singles.tile([128, JPH, PADH, PADW], BF16, name=f"a0_{half}")
            a1 = singles.tile([128, JPH, PADH, W], BF16, name=f"a1_{half}")
            for g in range(2):
                nc.sync.dma_start(out=a0[g * 64:(g + 1) * 64],
                                  in_=xa[:, g * 8 + j0:g * 8 + j0 + JPH])
                nc.sync.dma_start(out=a1[g * 64:(g + 1) * 64],
                                  in_=xb[:, g * 8 + j0:g * 8 + j0 + JPH])
            aux0s.append(a0)
            aux1s.append(a1)

        # ---- weights (needed by the first tap tiles) ----
        w_sb = singles.tile([128, CO * 9], F32)
        neg_w_sb = singles.tile([128, CO * 9], F32)
        w_src = w.rearrange("co ci kh kw -> ci co (kh kw)")
        wneg_src = wneg.rearrange("co ci kh kw -> ci co (kh kw)")
        for g in range(2):
            nc.sync.dma_start(
                out=w_sb[g * 64:(g + 1) * 64, :].rearrange(
                    "p (co t) -> p co t", t=9), in_=w_src)
            nc.sync.dma_start(
                out=neg_w_sb[g * 64:(g + 1) * 64, :].rearrange(
                    "p (co t) -> p co t", t=9), in_=wneg_src)

        # ---- PE selector weights (bf16, exact): -2 * group-reduce for ALL
        # streams (box too; fixed up with a -0.5 scale in stage 2) so the
        # stationary weights never change.
        selm2 = singles.tile([128, 2], BF16)
        nc.vector.memset(selm2[:, :], 0.0)
        nc.vector.memset(selm2[0:64, 0:1], -2.0)
        nc.vector.memset(selm2[64:128, 1:2], -2.0)
        eps_t = singles.tile([128, 1], F32)
        nc.vector.memset(eps_t[:, :], BN_EPS)

        gam = singles.tile([128, 1], F32)
        bet = singles.tile([128, 1], F32)
        for co in range(CO):
            nc.sync.dma_start(out=gam[co * 16:(co + 1) * 16, :],
                              in_=gamma[co:co + 1].partition_broadcast(16))
            nc.sync.dma_start(out=bet[co * 16:(co + 1) * 16, :],
                              in_=beta[co:co + 1].partition_broadcast(16))
        selcor = singles.tile([128, 128], F32)      # replicated stats selector
        nc.sync.dma_start(out=selcor[:, :], in_=selcor_in[:, :])

        def tap_src(half, kh, kw):
            """bf16 view of the (kh,kw)-shifted window, 4B-aligned."""
            if kw == 1:
                return aux1s[half][:, :, kh:kh + H, 0:W]
            return aux0s[half][:, :, kh:kh + H, kw:kw + W]

        def box_rhs(half, kh, kw, b):
            a, hb = divmod(b, 2)
            if kw == 1:
                return aux1s[half][:, a, kh + hb * 16:kh + hb * 16 + 16, 0:W]
            return aux0s[half][:, a, kh + hb * 16:kh + hb * 16 + 16,
                               kw:kw + W]

        # conv scratch in DRAM (partition-crossing redistribution)
        dpool = ctx.enter_context(tc.tile_pool(name="dram", bufs=1,
                                               space="DRAM"))
        conv_d = dpool.tile([CO, 2, NH, JPH, HW], F32)
        box_d = dpool.tile([2, NH, JPH, HW], F32)

        # stage-2 reload targets, loaded incrementally
        cs_rl = spool.tile([128, HW], F32)      # [(co,n), hw] = -2*sum(relu)
        box_rl = spool.tile([128, HW], F32)     # BoxX broadcast per co

        # ---- stage 1: rounds of up to 4 concurrent streams ----
        acc = 0.0
        co_last_round = {}
        box_last_round = 0
        for s, (kind, co, h) in enumerate(streams):
            if kind == "co":
                co_last_round[co] = s // 4
            else:
                box_last_round = s // 4
        HB = TCOLS // 2  # psum half-tile cols (2048)
        for r in range(n_rounds):
            rs = streams[4 * r:4 * r + 4]
            # two psum half-tiles (images 0-1 / 2-3): next round can start
            # in half A while half B is still evacuating
            psA = pspool.tile([128, HB], F32, tag="ps", name=f"psA_{r}")
            psB = pspoolB.tile([128, HB], F32, tag="psb", name=f"psB_{r}")

            def emit_mm(tap, b_range):
                kh, kw = divmod(tap, 3)
                for b in b_range:
                    ps = psA if b < NB // 2 else psB
                    col = (b % (NB // 2)) * 512
                    for j, (kind, co, h) in enumerate(rs):
                        if kind == "co":
                            rhs = t_tiles[j][:, b * 512:(b + 1) * 512]
                        else:
                            rhs = box_rhs(h, kh, kw, b)
                        nc.tensor.matmul(
                            ps[32 * j:32 * j + 2, col:col + 512],
                            lhsT=selm2[:, :], rhs=rhs,
                            start=(tap == 0), stop=(tap == 8),
                            tile_position=(0, 32 * j))

            t_tiles = {}
            for tap in range(9):
                kh, kw = divmod(tap, 3)
                for j, (kind, co, h) in enumerate(rs):
                    if kind != "co":
                        continue
                    t = tpool.tile([128, JPH, H, W], BF16, tag="t",
                                   name=f"t_{r}_{j}_{tap}")
                    src = tap_src(h, kh, kw)
                    k = co * 9 + tap
                    acc += act_frac
                    if acc >= 1.0:
                        acc -= 1.0
                        nc.scalar.activation(
                            out=t[:, :, :, :], in_=src,
                            func=mybir.ActivationFunctionType.Relu,
                            bias=neg_w_sb[:, k:k + 1], scale=1.0)
                    else:
                        nc.vector.tensor_scalar(
                            out=t[:, :, :, :], in0=src,
                            scalar1=w_sb[:, k:k + 1], scalar2=0.0,
                            op0=mybir.AluOpType.subtract,
                            op1=mybir.AluOpType.max)
                    t_tiles[j] = t.rearrange("p a h w -> p (a h w)")
                if tap < 8:
                    emit_mm(tap, range(NB))
                else:
                    emit_mm(tap, range(NB // 2))

            csA = cpool.tile([128, HB], F32, tag="cs", name=f"csA_{r}")
            nc.scalar.copy(csA[:, :], psA[:, :])
            emit_mm(8, range(NB // 2, NB))
            csB = cpool.tile([128, HB], F32, tag="cs", name=f"csB_{r}")
            nc.scalar.copy(csB[:, :], psB[:, :])
            for j, (kind, co, h) in enumerate(rs):
                for half_i, cs in enumerate((csA, csB)):
                    strip = cs[32 * j:32 * j + 2, :].rearrange(
                        "p (a hw) -> p a hw", hw=HW)
                    asl = slice(half_i * 2, half_i * 2 + 2)
                    if kind == "co":
                        nc.sync.dma_start(out=conv_d[co, :, h, asl, :],
                                          in_=strip)
                    else:
                        nc.sync.dma_start(out=box_d[:, h, asl, :], in_=strip)
            # incremental stage-2 reloads as soon as a channel completes
            for co2, lr in co_last_round.items():
                if lr == r:
                    nc.sync.dma_start(
                        out=cs_rl[co2 * 16:(co2 + 1) * 16, :],
                        in_=conv_d[co2].rearrange("g h j w -> (g h j) w"))
            if r == box_last_round:
                for c2 in range(CO):
                    nc.sync.dma_start(
                        out=box_rl[c2 * 16:(c2 + 1) * 16, :],
                        in_=box_d.rearrange("g h j w -> (g h j) w"))

        # ---- stage 2: BN stats + affine + relu ----
        # conv0 = cs_rl - 0.5*box_rl = conv + S_w (the S_w shift cancels in
        # BN: using mean0 = mean(conv0) everywhere is equivalent), with free
        # per-partition row sums via accum_out
        conv_rl = spool.tile([128, HW], F32)
        psums = spool.tile([128, 1], F32)
        nc.vector.scalar_tensor_tensor(
            out=conv_rl[:, :], in0=box_rl[:, :], scalar=-0.5,
            in1=cs_rl[:, :], op0=mybir.AluOpType.mult,
            op1=mybir.AluOpType.add, accum_out=psums[:, :])

        # second moment per partition, independent of mean (runs right after
        # conv_rl, before the stats matmuls)
        sq = spool.tile([128, HW], F32)
        pssq = spool.tile([128, 1], F32)
        nc.vector.scalar_tensor_tensor(
            out=sq[:, :], in0=conv_rl[:, :], scalar=1.0, in1=conv_rl[:, :],
            op0=mybir.AluOpType.mult, op1=mybir.AluOpType.mult,
            accum_out=pssq[:, :])
        # selcor carries 1/(N*HW), so these give mean / E[c^2] directly
        mean_ps = pspool.tile([128, 1], F32, tag="ps")
        nc.tensor.matmul(mean_ps[:, :], lhsT=selcor[:, :], rhs=psums[:, :],
                         start=True, stop=True)
        ps2 = pspoolB.tile([128, 1], F32, tag="psb")
        nc.tensor.matmul(ps2[:, :], lhsT=selcor[:, :], rhs=pssq[:, :],
                         start=True, stop=True)
        mean = spool.tile([128, 1], F32)
        nc.vector.tensor_copy(out=mean[:, :], in_=mean_ps[:, :])
        # var = E[c^2] - mean^2
        m2 = spool.tile([128, 1], F32)
        nc.vector.tensor_mul(m2[:, :], mean[:, :], mean[:, :])
        var = spool.tile([128, 1], F32)
        nc.vector.tensor_sub(var[:, :], ps2[:, :], m2[:, :])
        std = spool.tile([128, 1], F32)
        nc.scalar.activation(out=std[:, :], in_=var[:, :],
                             func=mybir.ActivationFunctionType.Sqrt,
                             bias=eps_t[:, :], scale=1.0)
        rstd = spool.tile([128, 1], F32)
        nc.vector.reciprocal(out=rstd[:, :], in_=std[:, :])
        a_t = spool.tile([128, 1], F32)
        nc.vector.tensor_mul(a_t[:, :], gam[:, :], rstd[:, :])
        ma = spool.tile([128, 1], F32)
        nc.vector.tensor_mul(ma[:, :], mean[:, :], a_t[:, :])
        b_t = spool.tile([128, 1], F32)
        nc.vector.tensor_sub(b_t[:, :], bet[:, :], ma[:, :])

        outt = spool.tile([128, HW], F32)
        if debug_out == "conv":
            nc.vector.tensor_copy(out=outt[:, :], in_=conv_rl[:, :])
        elif debug_out == "cs":
            nc.vector.tensor_copy(out=outt[:, :], in_=cs_rl[:, :])
        elif debug_out == "box":
            nc.vector.tensor_copy(out=outt[:, :], in_=box_rl[:, :])
        else:
            nc.scalar.activation(out=outt[:, :], in_=conv_rl[:, :],
                                 func=mybir.ActivationFunctionType.Relu,
                                 bias=b_t[:, :], scale=a_t[:, :])
        out_r = out.rearrange("n co h w -> co n (h w)")
        for co in range(CO):
            nc.sync.dma_start(out=out_r[co], in_=outt[co * 16:(co + 1) * 16, :])

    split_multiwaits(nc)
    return nc


def make_in_maps(x, weight, gamma, beta):
    import ml_dtypes
    x = np.ascontiguousarray(x, dtype=np.float32)
    weight = np.ascontiguousarray(weight, dtype=np.float32)
    gamma = np.ascontiguousarray(gamma, dtype=np.float32)
    beta = np.ascontiguousarray(beta, dtype=np.float32)
    # pre-padded, ci-major, bf16 copies of x (xb shifted one element in w)
    xpad = np.zeros((CI, N, PADH, PADW), np.float32)
    xpad[:, :, 1:H + 1, 1:W + 1] = x.transpose(1, 0, 2, 3)
    xa = np.ascontiguousarray(xpad.astype(ml_dtypes.bfloat16))
    xb = np.ascontiguousarray(xa[:, :, :, 1:1 + W])
    selcor = np.zeros((128, 128), np.float32)
    for c in range(CO):
        selcor[c * 16:(c + 1) * 16, c * 16:(c + 1) * 16] = 1.0 / (N * H * W)
    maps = []
    for c in range(8):
        sl = slice(c * CO, (c + 1) * CO)
        wsl = np.ascontiguousarray(weight[sl])
        maps.append({
            "xa": xa,
            "xb": xb,
            "w": wsl,
            "wneg": np.ascontiguousarray(-wsl),
            "swneg": np.ascontiguousarray(-wsl.sum(axis=(1, 2, 3))),
            "gamma": np.ascontiguousarray(gamma[sl]),
            "beta": np.ascontiguousarray(beta[sl]),
            "selcor": selcor,
        })
    return maps


def assemble(results):
    return np.concatenate([r["out"] for r in results], axis=1)


# ---------------------------------------------------------------------------
# Harness entry point: full inputs in, full output out.
# Sharding: output channels co split 8 ways (8 channels per NeuronCore);
# BN statistics are over the full batch, which each core owns for its
# channels, so no collectives are needed.
# ---------------------------------------------------------------------------
from concourse.bass_utils import run_bass_kernel_spmd

_NC_CACHE = None


def _get_nc():
    global _NC_CACHE
    if _NC_CACHE is None:
        _NC_CACHE = build_nc()
    return _NC_CACHE


def kernel(x, weight, gamma, beta):
    nc = _get_nc()
    in_maps = make_in_maps(np.asarray(x), np.asarray(weight),
                           np.asarray(gamma), np.asarray(beta))
    res = run_bass_kernel_spmd(nc, in_maps, core_ids=list(range(8)))
    return assemble(res.results)
